# revision 2
# baseline (speedup 1.0000x reference)
"""Trainium2 Bass kernel for ChunkedTGnnModel — fp8 DoubleRow redesign.

Math (per temporal chunk, flattened to a [128000, 64] slab whose
consecutive 1000-row blocks each see the same dense normalized adjacency
A_hat [1000 x 1000]):

    out = relu(blockdiag(A_hat) @ (slab @ W1) + b1)   (layer 2 same)

Layer ordering is W-fold FIRST (P = slab @ W), then the A-type matmul,
so every layer epilogue lands on the feature-major side where bias is
per-partition and fuses with relu into one ACT op.

The A-type (dominant cost) runs as fp8e4m3 DoubleRow matmuls (0.5
cycles/row, 2 k-tiles of 125 rows per instruction) with a 3-product
error-compensation split:  A@P ~= Ah@Ph + Al@Ph + Ah@Pl  where
Xh = fp8(X), Xl = fp8(X - Xh).  Dropped Al@Pl term is O(eps^2).

Sharding: 8 cores = 4 chunks x 2 node-halves; each core owns 64 blocks
(32 block-pairs) of [1000, 64]. Per pair the W-fold stacks both blocks:
xT [(2b,f)=128 part, 1000 rows], Wbig = blockdiag(W) [128, 128].

Pipeline per iteration p: W2[p-1], W1[p+1] (fp16, PE), then A1[p],
A2[p-1] (fp8 DR, PE) covering the ACT/DVE hi/lo split latency of the
just-produced P tiles. PSUM: 4 tags x 2 banks = 8 banks exactly.
"""
import sys
import numpy as np
import ml_dtypes

sys.path.insert(0, '/opt/trn_rl_repo')

import concourse.bass as bass  # noqa: E402
import concourse.bacc as bacc  # noqa: E402
import concourse.mybir as mybir  # noqa: E402
import concourse.tile as tile  # noqa: E402
from concourse.bass_utils import run_bass_kernel_spmd  # noqa: E402

N, T, D = 1000, 512, 64
CS = 128                 # timesteps per chunk
NCORES = 8
PAIRS = 32               # block-pairs per core (64 blocks of 1000 rows)
F8 = ml_dtypes.float8_e4m3
DR = mybir.MatmulPerfMode.DoubleRow

_prog = None
LAST_RESULTS = None


def _build_program(skip=frozenset()):
    nc = bacc.Bacc(None)
    xt = nc.declare_dram_parameter("xt", [PAIRS, 128, N], mybir.dt.float16,
                                   isOutput=False)
    ath = nc.declare_dram_parameter("ath", [128, 8 * N], mybir.dt.float8e4,
                                    isOutput=False)
    atl = nc.declare_dram_parameter("atl", [128, 8 * N], mybir.dt.float8e4,
                                    isOutput=False)
    wt1 = nc.declare_dram_parameter("wt1", [128, 128], mybir.dt.float16,
                                    isOutput=False)
    wt2 = nc.declare_dram_parameter("wt2", [128, 128], mybir.dt.float16,
                                    isOutput=False)
    bs1 = nc.declare_dram_parameter("bs1", [128, 1], mybir.dt.float32,
                                    isOutput=False)
    bs2 = nc.declare_dram_parameter("bs2", [128, 1], mybir.dt.float32,
                                    isOutput=False)
    xout = nc.declare_dram_parameter("xout", [PAIRS, 128, N],
                                     mybir.dt.float16, isOutput=True)

    with tile.TileContext(nc) as tc:
        with tc.tile_pool(name="const", bufs=1) as cpool, \
             tc.tile_pool(name="work", bufs=2) as wpool, \
             tc.tile_pool(name="psA", bufs=1, space="PSUM") as psA, \
             tc.tile_pool(name="psB", bufs=1, space="PSUM") as psB:

            st = {}

            def load_xt(p):
                t = wpool.tile([128, N], mybir.dt.float16, name="xt_t",
                               tag="xt_t")
                if "indma" not in skip:
                    nc.sync.dma_start(t[:, :], xt[p, :, :])
                else:
                    nc.sync.dma_start(t[0:2, 0:2], xt[p, 0:2, 0:2])
                st[(p, 'xt')] = t

            def wfold(p, li):
                """8 fp16 matmuls: lhsT = input chunks, rhs = Wbig."""
                src = st.pop((p, 'xt')) if li == 0 else st.pop((p, 'h1'))
                pps = [psA.tile([128, 512], mybir.dt.float32,
                                name=f"pps{li}_{t}", tag=f"pps{li}_{t}")
                       for t in range(2)]
                ww = 128 if "wfold" not in skip else 4
                for c in range(8):
                    nc.tensor.matmul(
                        pps[c // 4][0:125,
                                    128 * (c % 4):128 * (c % 4) + ww],
                        src[:, 125 * c:125 * c + 125], wt_t[li][:, 0:ww],
                        start=True, stop=True)
                st[(p, f'pps{li}')] = pps

            def split(p, li):
                """P psum -> fp8 hi/lo sbuf tiles [128, 8, 128]."""
                pps = st.pop((p, f'pps{li}'))
                nb = 3 if li == 0 else 2   # layer-1 tiles live 2 iters ahead
                hi = wpool.tile([128, 8, 128], mybir.dt.float8e4,
                                name=f"hi{li}", tag=f"hi{li}", bufs=nb)
                lo = wpool.tile([128, 8, 128], mybir.dt.float8e4,
                                name=f"lo{li}", tag=f"lo{li}", bufs=nb)
                hv = hi.rearrange("p k w -> p (k w)")
                lv = lo.rearrange("p k w -> p (k w)")
                # hi-casts on ACT, lo-subs on DVE (chained per half so the
                # first lo lands quickly)
                tw = 512 if "split" not in skip else 4
                for t in range(2):
                    nc.scalar.copy(hv[0:125, 512 * t:512 * t + tw],
                                   pps[t][0:125, 0:tw])
                    nc.vector.tensor_tensor(
                        lv[0:125, 512 * t:512 * t + tw],
                        pps[t][0:125, 0:tw],
                        hv[0:125, 512 * t:512 * t + tw],
                        op=mybir.AluOpType.subtract)
                st[(p, f'sp{li}')] = (hi, lo)

            def atype(p, li):
                """48 DR matmuls -> G psum, 2 banks x (2 chunks of 250)."""
                hi, lo = st.pop((p, f'sp{li}'))
                gps = [psB.tile([128, 512], mybir.dt.float32,
                                name=f"gps{li}_{t}", tag=f"gps{li}_{t}")
                       for t in range(2)]
                # A-hi products first: at the prologue A-lo arrives last,
                # and the lo-sub DVE chain lands well before matmul 17
                # A-hi products first: at the prologue A-lo arrives last,
                # and the lo-sub DVE chain lands well before matmul 17
                prods = ((hi, ath_t), (lo, ath_t), (hi, atl_t))
                aw = 250 if "atype" not in skip else 4
                nmm = [0, 0]
                for (lt, rt) in prods:
                    for bank in range(2):
                        for ci in range(2):
                            n0 = 500 * bank + 250 * ci
                            for m in range(4):
                                nc.tensor.matmul(
                                    gps[bank][:, 250 * ci:250 * ci + aw],
                                    lt[0:125, 2 * m:2 * m + 2, :],
                                    rt[0:125, 2 * m:2 * m + 2, n0:n0 + aw],
                                    start=(nmm[bank] == 0),
                                    stop=(nmm[bank] == 23),
                                    perf_mode=DR)
                                nmm[bank] += 1
                st[(p, f'g{li}')] = gps

            def relu(p, li):
                """Fused bias+relu, feature-major out (fp16)."""
                gps = st.pop((p, f'g{li}'))
                name = 'h1' if li == 0 else 'ot'
                t = wpool.tile([128, N], mybir.dt.float16, name=name,
                               tag=name)
                rw = 500 if "relu" not in skip else 4
                for bank in range(2):
                    if li == 0:
                        nc.scalar.activation(
                            t[:, 500 * bank:500 * bank + rw],
                            gps[bank][:, 0:rw],
                            mybir.ActivationFunctionType.Relu,
                            bias=bs_t[li][:, :])
                    else:
                        # layer-2 relu on DVE: max(in + b, 0) fused
                        nc.vector.tensor_scalar(
                            t[:, 500 * bank:500 * bank + rw],
                            gps[bank][:, 0:rw],
                            bs_t[li][:, :], 0.0,
                            op0=mybir.AluOpType.add,
                            op1=mybir.AluOpType.max)
                st[(p, name)] = t

            def store(p):
                t = st.pop((p, 'ot'))
                if "outdma" not in skip:
                    # per-bank halves; last pairs use two parallel HWDGE
                    # queues to shorten the drain tail
                    if p >= PAIRS - 2:
                        nc.sync.dma_start(xout[p, :, 0:500], t[:, 0:500])
                        nc.scalar.dma_start(xout[p, :, 500:1000],
                                            t[:, 500:1000])
                    else:
                        nc.gpsimd.dma_start(xout[p, :, 0:500], t[:, 0:500])
                        nc.gpsimd.dma_start(xout[p, :, 500:1000],
                                            t[:, 500:1000])
                else:
                    nc.gpsimd.dma_start(xout[p, 0:2, 0:2], t[0:2, 0:2])

            # ---- prologue: spread loads over all 4 DMA queues so the
            # A-matrix (critical for A1[0]) and W/x (critical for W1[0])
            # arrive in parallel ----
            ath_t = cpool.tile([128, 8, N], mybir.dt.float8e4, name="ath_t")
            atl_t = cpool.tile([128, 8, N], mybir.dt.float8e4, name="atl_t")
            ath_v = ath.rearrange("p (k n) -> p k n", k=8)
            atl_v = atl.rearrange("p (k n) -> p k n", k=8)
            wt_t = [cpool.tile([128, 128], mybir.dt.float16, name=f"wt{li}")
                    for li in range(2)]
            bs_t = [cpool.tile([128, 1], mybir.dt.float32, name=f"bst{li}")
                    for li in range(2)]
            # SP/HWDGE: x0 and W1 lead the shared DMA device; A-matrix
            # halves ride both queues right behind; x1 before A-lo h2
            load_xt(0)
            nc.sync.dma_start(wt_t[0][:, :], wt1[:, :])
            nc.gpsimd.dma_start(ath_t[:, 0:4, :], ath_v[:, 0:4, :])
            nc.sync.dma_start(ath_t[:, 4:8, :], ath_v[:, 4:8, :])
            load_xt(1)
            nc.gpsimd.dma_start(atl_t[:, 0:4, :], atl_v[:, 0:4, :])
            nc.sync.dma_start(atl_t[:, 4:8, :], atl_v[:, 4:8, :])
            # consts for iter 1+: delay so they don't steal early DMA slots
            with tc.tile_wait_until(0.006):
                nc.scalar.dma_start(wt_t[1][:, :], wt2[:, :])
                nc.scalar.dma_start(bs_t[0][:, :], bs1[:, :])
                nc.scalar.dma_start(bs_t[1][:, :], bs2[:, :])
            # pre-fold pairs 0 and 1 while the A-matrix streams in
            wfold(0, 0)
            split(0, 0)
            wfold(1, 0)
            split(1, 0)
            load_xt(2)

            # ---- steady state (W1 lookahead = 2) ----
            for p in range(PAIRS + 1):
                if p >= 1:
                    wfold(p - 1, 1)       # needs relu1[p-1] (prev iter)
                    split(p - 1, 1)
                if p + 2 < PAIRS:
                    wfold(p + 2, 0)       # needs xt[p+2]
                    split(p + 2, 0)
                if p + 3 < PAIRS:
                    load_xt(p + 3)
                if p < PAIRS:
                    atype(p, 0)           # covers split latencies
                    relu(p, 0)
                if p >= 1:
                    atype(p - 1, 1)
                    relu(p - 1, 1)
                    store(p - 1)

    nc.compile()
    return nc


def _host_prep(x, edge_index, W1, b1, W2, b2):
    x = np.ascontiguousarray(np.asarray(x, dtype=np.float32))
    ei = np.asarray(edge_index)
    row, col = ei[0], ei[1]
    deg = np.zeros(N, np.float32)
    np.add.at(deg, col, 1.0)
    deg += 1.0
    dinv = (1.0 / np.sqrt(deg)).astype(np.float32)
    A = np.zeros((N, N), np.float32)
    np.add.at(A, (col, row), (dinv[row] * dinv[col]).astype(np.float32))
    A[np.arange(N), np.arange(N)] += dinv * dinv
    AT = np.ascontiguousarray(A.T)                      # [src, dst]
    ATh = AT.astype(F8)
    ATl = (AT - ATh.astype(np.float32)).astype(F8)

    def pack_at(at8):
        pk = np.zeros((128, 8, N), F8)
        pk8 = at8.reshape(8, 125, N)
        pk[0:125] = pk8.transpose(1, 0, 2)
        return np.ascontiguousarray(pk.reshape(128, 8 * N))

    wts = []
    for W in (W1, W2):
        wt = np.zeros((128, 128), np.float16)
        wt[:64, :64] = np.asarray(W).astype(np.float16)
        wt[64:, 64:] = np.asarray(W).astype(np.float16)
        wts.append(wt)
    bss = [np.ascontiguousarray(np.tile(np.asarray(b, np.float32), 2)
                                .reshape(128, 1)) for b in (b1, b2)]

    x16 = x.astype(np.float16)
    xts = []
    for k in range(NCORES):
        c, hf = k // 2, k % 2
        slab = x16[500 * hf:500 * hf + 500,
                   128 * c:128 * (c + 1), :].reshape(64000, D)
        # [pair, (2b, f), row-in-block]
        xt = slab.reshape(PAIRS, 2, N, D).transpose(0, 1, 3, 2)
        xts.append(np.ascontiguousarray(xt.reshape(PAIRS, 128, N)))
    return pack_at(ATh), pack_at(ATl), wts, bss, xts


def kernel(x, edge_index, W1, b1, W2, b2):
    global _prog, LAST_RESULTS
    if _prog is None:
        _prog = _build_program()
    nc = _prog

    ATh, ATl, wts, bss, xts = _host_prep(x, edge_index, W1, b1, W2, b2)
    in_maps = [{"xt": xts[k], "ath": ATh, "atl": ATl,
                "wt1": wts[0], "wt2": wts[1],
                "bs1": bss[0], "bs2": bss[1]} for k in range(NCORES)]

    LAST_RESULTS = run_bass_kernel_spmd(nc, in_maps,
                                        core_ids=list(range(NCORES)))

    out = np.empty((N, T, D), np.float32)
    for k in range(NCORES):
        c, hf = k // 2, k % 2
        ot = LAST_RESULTS.results[k]["xout"]          # [PAIRS, 128, N] f16
        # [pair, (2b, f), row] -> rows [pair*2+b]*1000+row, feat f
        slab = ot.reshape(PAIRS, 2, D, N).transpose(0, 1, 3, 2) \
                 .reshape(64000, D).astype(np.float32)
        out[500 * hf:500 * hf + 500, 128 * c:128 * (c + 1), :] = \
            slab.reshape(500, CS, D)
    return out
